# revision 1
# baseline (speedup 1.0000x reference)
"""2-layer GCN (GraphConv x2 + mean-pool + linear) on 8 TRN2 NeuronCores.

Strategy (dead-code-eliminated 1D partition):
  The output is mean(h2[:order+1]) @ Wl + bl, so only h2 rows 0..order matter.
  Those depend on layer-2 edges with dst <= order (~12.8k of 1.25M), whose
  unique srcs (~12.3k nodes) are the only h1 rows ever needed. Layer-1
  therefore only aggregates the ~158k edges incident to those nodes.

  - The order+1 pool rows are split into 128-row blocks, round-robin across
    8 cores (1024 -> exactly one block per core). Each core owns the h1 rows
    (compact-renumbered, degree-balanced into 128-row blocks) needed by its
    pool edges; rows needed by several cores are recomputed on each (~4%).
    No cross-core collective at all: each core emits a partial [64] output
    (its pool block's mean through W2/Wl), summed on the host (+ bl).
  - Layer-1 X rows come from a channel-major SBUF-RESIDENT feat table
    (featp [128, 50k] bf16: partitions 0-63 = channels x node-pairs for
    nodes [0,50k), 64-127 for [50k,100k)), fetched by ONE gpsimd ap_gather
    ucode op per 128-dst block (d=2 node-pair elements keep the int16 index
    space and 4-byte ucode granularity legal). This replaces 169 serialized
    SWDGE indirect DMAs (~1.2us each on the Pool queue -- the HW DGE consumes
    only one offset per dest partition run, so batched offset APs can't
    amortize it) with 13 cheap Q7 ops. Gathered cols are PE-transposed in
    [64,128] slices into 64-real-edge chunks (PSUM bf16 staging, DVE/ACT
    alternating copy to SBUF), then aggT[64, dsts] += T^T @ M with
    M[row, d] = (dst==d)*w_e precomputed host-side (ghost/pad rows zero);
    narrow W-col windows for all but each block's first chunk.
  - Layer-2's 13 gathers stay on SWDGE indirect DMA (h1t rows from DRAM).
  - Transform epilogue uses the reversed matmul orientation
    z[dst, ch] = (aggT_aug)^T-free: matmul(lhsT=agg_aug[65,128],
    rhs=Waug[65,64]) with an appended ones-row carrying the bias, so h1
    lands directly in row-major [dst, 64] for the layer-2 gather: no
    transpose instructions. Pool masking/mean folds into a per-core msk
    column used as the pooling matmul rhs.
  - bf16 payloads (feat gather, M, h1) halve DMA bytes; PSUM accumulates
    fp32; the tiny tail (pool/Wl) stays fp32.
  - degrees / normalization / edge grouping / M build are host-side index
    prep (numpy); all feature math runs on device.
"""

import numpy as np

N_NODES = 100_000
N_EDGES = 1_250_000
C = 64
N_CORES = 8
NEG_SLOPE = 0.01
BLK = 128          # dst nodes per PSUM block
CHUNK = 128        # edges per matmul chunk (PE K dim)
GOP = 64           # chunks per batched indirect-DMA gather

_cache = {}


def _build(meta):
    import concourse.bass as bass
    import concourse.bacc as bacc
    import concourse.mybir as mybir
    import concourse.tile as tile

    f32 = mybir.dt.float32
    i32 = mybir.dt.int32
    dt = mybir.dt.bfloat16 if meta.get("bf16", True) else mybir.dt.float32

    n_nodes = meta["n_nodes"]
    nb1 = meta["nb1"]                  # layer-1 compact-dst blocks
    NI = meta["NI"]                    # gathered idxs per (block, half)
    NIc = NI // 64                     # transposed chunks per (block, half)
    cpb1 = 2 * NIc                     # chunks per layer-1 block
    num_elems = meta["num_elems"]      # node-pairs per table half
    W1w = meta["W1w"]                  # narrow window width, layer 1
    woff1 = meta["woff1"]              # per (b, jb>=1) window offsets
    npb = meta["npb"]                  # pool blocks per core
    cpb2 = meta["cpb2"]
    W2w = meta["W2w"]
    woff2 = meta["woff2"]
    reps = meta.get("reps", 1)
    nq = meta.get("nq", 1)

    n2_chunks = npb * cpb2
    h1_rows = nb1 * BLK
    i16 = mybir.dt.int16

    nc = bacc.Bacc(None, target_bir_lowering=False, num_swdge_queues=nq)

    featp = nc.declare_dram_parameter("featp", [128, num_elems * 2], dt,
                                      isOutput=False)
    idxg = nc.declare_dram_parameter("idxg", [128, nb1 * (NI // 16)], i16,
                                     isOutput=False)
    m1f = nc.declare_dram_parameter("m1f", [128, nb1 * BLK], dt, isOutput=False)
    if cpb1 > 1:
        m1n = nc.declare_dram_parameter("m1n", [128, nb1 * (cpb1 - 1) * W1w], dt,
                                        isOutput=False)
    idx2 = nc.declare_dram_parameter("idx2", [128, n2_chunks], i32, isOutput=False)
    m2f = nc.declare_dram_parameter("m2f", [128, npb * BLK], dt, isOutput=False)
    if cpb2 > 1:
        m2n = nc.declare_dram_parameter("m2n", [128, npb * (cpb2 - 1) * W2w], dt,
                                        isOutput=False)
    wts = nc.declare_dram_parameter("wts", [65, 3 * 64], f32, isOutput=False)
    msk = nc.declare_dram_parameter("msk", [128, npb], f32, isOutput=False)
    outp = nc.declare_dram_parameter("out", [64], f32, isOutput=True)

    with tile.TileContext(nc) as tc:
        with (
            tc.tile_pool(name="dram", bufs=1, space="DRAM") as dram,
            tc.tile_pool(name="res", bufs=1) as res,
            tc.tile_pool(name="gbuf", bufs=3) as gpool,
            tc.tile_pool(name="g2buf", bufs=16) as g2pool,
            tc.tile_pool(name="tbuf", bufs=4) as tpool,
            tc.tile_pool(name="ep", bufs=3) as ep,
            tc.tile_pool(name="psA", bufs=2, space="PSUM") as psA,
            tc.tile_pool(name="psT", bufs=2, space="PSUM") as psT,
            tc.tile_pool(name="psB", bufs=2, space="PSUM") as psB,
            tc.tile_pool(name="psC", bufs=1, space="PSUM") as psC,
        ):
            h1t = dram.tile([h1_rows, C], dt)
            # ---- resident metadata / weights ----
            featp_t = res.tile([128, num_elems * 2], dt)
            idxg_t = res.tile([128, nb1 * (NI // 16)], i16)
            m1f_t = res.tile([128, nb1 * BLK], dt)
            idx2_t = res.tile([128, n2_chunks], i32)
            m2f_t = res.tile([128, npb * BLK], dt)
            wts_t = res.tile([65, 3 * 64], f32)
            msk_t = res.tile([128, npb], f32)
            nc.sync.dma_start(out=featp_t[:], in_=featp[:, :])
            nc.sync.dma_start(out=idxg_t[:], in_=idxg[:, :])
            nc.sync.dma_start(out=m1f_t[:], in_=m1f[:, :])
            nc.sync.dma_start(out=idx2_t[:], in_=idx2[:, :])
            nc.sync.dma_start(out=m2f_t[:], in_=m2f[:, :])
            nc.sync.dma_start(out=wts_t[:], in_=wts[:, :])
            nc.sync.dma_start(out=msk_t[:], in_=msk[:, :])
            if cpb1 > 1:
                m1n_t = res.tile([128, nb1 * (cpb1 - 1) * W1w], dt)
                nc.sync.dma_start(out=m1n_t[:], in_=m1n[:, :])
            if cpb2 > 1:
                m2n_t = res.tile([128, npb * (cpb2 - 1) * W2w], dt)
                nc.sync.dma_start(out=m2n_t[:], in_=m2n[:, :])
            # low-precision copies of the augmented transforms for the PE
            w1a = res.tile([65, 64], dt)
            w2a = res.tile([65, 64], dt)
            nc.vector.tensor_copy(out=w1a[:], in_=wts_t[:, 0:64])
            nc.vector.tensor_copy(out=w2a[:], in_=wts_t[:, 64:128])
            wl_t = res.tile([64, 64], f32)
            nc.vector.tensor_copy(out=wl_t[:], in_=wts_t[0:64, 128:192])
            from concourse.masks import make_identity
            identt = res.tile([128, 128], dt)
            make_identity(nc, identt[:])

            def m1_slice(b, jb):
                if jb == 0:
                    return m1f_t[:, b * BLK:(b + 1) * BLK], 0, BLK
                s = (b * (cpb1 - 1) + (jb - 1)) * W1w
                return m1n_t[:, s:s + W1w], woff1[b * (cpb1 - 1) + jb - 1], W1w

            def m2_slice(b, jb):
                if jb == 0:
                    return m2f_t[:, b * BLK:(b + 1) * BLK], 0, BLK
                s = (b * (cpb2 - 1) + (jb - 1)) * W2w
                return m2n_t[:, s:s + W2w], woff2[b * (cpb2 - 1) + jb - 1], W2w

            def body():
                # ---- layer 1 ----
                # one ap_gather per block: channel-major SBUF-resident feat
                # table (node-pair packed, halves on partition groups 0-63 /
                # 64-127); gathered [128, NI, 2] then PE-transposed into
                # 64-real-edge chunks (ghost rows get zero M weight)
                for b in range(nb1):
                    gat = gpool.tile([128, NI * 2], dt, tag="gat")
                    nc.gpsimd.ap_gather(
                        out_ap=gat[:], in_ap=featp_t[:],
                        idxs_ap=idxg_t[:, b * (NI // 16):(b + 1) * (NI // 16)],
                        channels=128, num_elems=num_elems, d=2, num_idxs=NI)
                    acc = psA.tile([64, BLK], f32, tag="acc")
                    for c0 in range(0, cpb1, 2):
                        # two transposes stage into one PSUM tile; a single
                        # double-width DVE/ACT copy halves copy instruction
                        # overhead (cpb1 = 2*NIc is always even)
                        pst = psT.tile([128, 128], dt, tag="pst")
                        for ci in (c0, c0 + 1):
                            h, k = divmod(ci, NIc)
                            nc.tensor.transpose(
                                out=pst[:, (ci - c0) * 64:(ci - c0 + 1) * 64],
                                in_=gat[h * 64:(h + 1) * 64,
                                        k * 128:(k + 1) * 128],
                                identity=identt[h * 64:(h + 1) * 64,
                                                h * 64:(h + 1) * 64])
                        T = tpool.tile([128, 128], dt, tag="T")
                        if (c0 // 2) % 2:
                            nc.scalar.activation(
                                out=T[:], in_=pst[:],
                                func=mybir.ActivationFunctionType.Copy)
                        else:
                            nc.vector.tensor_copy(out=T[:], in_=pst[:])
                        for ci in (c0, c0 + 1):
                            m, off, w = m1_slice(b, ci)
                            nc.tensor.matmul(
                                out=acc[:, off:off + w],
                                lhsT=T[:, (ci - c0) * 64:(ci - c0 + 1) * 64],
                                rhs=m,
                                start=(ci == 0), stop=(ci == cpb1 - 1))
                    aug = ep.tile([65, BLK], dt, tag="aug")
                    nc.vector.tensor_copy(out=aug[0:64, :], in_=acc[:, :])
                    nc.vector.memset(aug[64:65, :], 1.0)
                    z = psB.tile([BLK, 64], f32, tag="z")
                    nc.tensor.matmul(out=z[:, :], lhsT=aug[:], rhs=w1a[:],
                                     start=True, stop=True)
                    hs = ep.tile([BLK, 64], dt, tag="hs")
                    nc.scalar.activation(out=hs[:], in_=z[:, :],
                                         func=mybir.ActivationFunctionType.Lrelu,
                                         scale=1.0, alpha=NEG_SLOPE)
                    nc.sync.dma_start(out=h1t[b * BLK:(b + 1) * BLK, :], in_=hs[:])

                # ---- layer 2 + pool + final linear (all core-local) ----
                g2tiles = []
                for c in range(n2_chunks):
                    g2 = g2pool.tile([128, C], dt, tag="g2")
                    ins = nc.gpsimd.indirect_dma_start(
                        out=g2[:], out_offset=None, in_=h1t[:, :],
                        in_offset=bass.IndirectOffsetOnAxis(
                            ap=idx2_t[:, c:c + 1], axis=0))
                    if nq > 1:
                        ins.ins.queue = f"qPoolDynamic{(c % nq) or ''}"
                    g2tiles.append(g2)
                pooled_ps = psC.tile([64, 1], f32, tag="pool")
                for b in range(npb):
                    acc = psA.tile([64, BLK], f32, tag="acc")
                    for jb in range(cpb2):
                        c = b * cpb2 + jb
                        m, off, w = m2_slice(b, jb)
                        nc.tensor.matmul(out=acc[:, off:off + w],
                                         lhsT=g2tiles[c][:], rhs=m,
                                         start=(jb == 0), stop=(jb == cpb2 - 1))
                    aug = ep.tile([65, BLK], dt, tag="aug")
                    nc.vector.tensor_copy(out=aug[0:64, :], in_=acc[:, :])
                    nc.vector.memset(aug[64:65, :], 1.0)
                    z = psB.tile([BLK, 64], f32, tag="z")
                    nc.tensor.matmul(out=z[:, :], lhsT=aug[:], rhs=w2a[:],
                                     start=True, stop=True)
                    h2 = ep.tile([BLK, 64], f32, tag="h2")
                    nc.scalar.activation(out=h2[:], in_=z[:, :],
                                         func=mybir.ActivationFunctionType.Lrelu,
                                         scale=1.0, alpha=NEG_SLOPE)
                    nc.tensor.matmul(out=pooled_ps[:, :], lhsT=h2[:],
                                     rhs=msk_t[:, b:b + 1],
                                     start=(b == 0), stop=(b == npb - 1))
                pooled_s = ep.tile([64, 1], f32, tag="pooled_s")
                nc.vector.tensor_copy(out=pooled_s[:], in_=pooled_ps[:, :])
                zf = psC.tile([64, 1], f32, tag="zf")
                nc.tensor.matmul(out=zf[:, :], lhsT=wl_t[:], rhs=pooled_s[:],
                                 start=True, stop=True)
                ofin = ep.tile([64, 1], f32, tag="ofin")
                nc.vector.tensor_copy(out=ofin[:], in_=zf[:, :])
                nc.sync.dma_start(out=outp[:, None], in_=ofin[:])

            for _ in range(reps):
                body()

    nc.compile()
    return nc


def _prep(src, dst, feat, W1, b1, W2, b2, Wl, bl, order, bf16=True):
    """Host-side index prep. Returns (meta, in_maps)."""
    src = np.asarray(src).astype(np.int64)
    dst = np.asarray(dst).astype(np.int64)
    n_nodes = feat.shape[0]
    pool_n = int(order) + 1

    out_deg = np.maximum(np.bincount(src, minlength=n_nodes), 1)
    in_deg = np.maximum(np.bincount(dst, minlength=n_nodes), 1)
    o_is = (out_deg.astype(np.float64) ** -0.5).astype(np.float32)
    i_is = (in_deg.astype(np.float64) ** -0.5).astype(np.float32)
    w_edge = o_is[src] * i_is[dst]

    # ---- pool edges -> owning core (pool block round-robin) ----
    n_pblocks = -(-pool_n // BLK)
    npb = -(-n_pblocks // N_CORES)
    e2 = np.nonzero(dst < pool_n)[0]
    pb = dst[e2] // BLK
    e2core = pb % N_CORES

    # ---- per-core needed h1 rows, compact ids (degree-balanced blocks) ----
    luts, kcs = [], []
    for core in range(N_CORES):
        uniq = np.unique(src[e2[e2core == core]])
        kcs.append(len(uniq))
        luts.append(uniq)
    nb1 = max(1, -(-max(kcs) // BLK))
    # balance rows AND their edge loads per (block, table-half): the gather
    # op count per block is driven by max(|A|, |B|) edges, so round-robin
    # each half's rows (degree-sorted) over blocks independently
    hn = (n_nodes + 1) // 2
    while True:
        ok = True
        lut_ids = []
        for core in range(N_CORES):
            uniq = luts[core]
            lut = np.full(n_nodes, -1, np.int64)
            fill = np.zeros(nb1, np.int64)
            for half in (0, 1):
                u = uniq[(uniq >= hn) == bool(half)]
                u = u[np.argsort(-in_deg[u], kind="stable")]
                for i, node in enumerate(u):
                    r, j = divmod(i, nb1)
                    b = j if r % 2 == 0 else nb1 - 1 - j  # snake order
                    lut[node] = b * BLK + fill[b]
                    fill[b] += 1
            if fill.max() > BLK:
                ok = False
                break
            lut_ids.append(lut)
        if ok:
            break
        nb1 += 1

    # ---- layer-1 edge streams per (core, block, half) ----
    half_nodes = (n_nodes + 1) // 2
    num_elems = (half_nodes + 1) // 2      # node-pairs per table half
    per_core_edges = []
    streams = {}
    lens = []
    for core in range(N_CORES):
        ld = lut_ids[core][dst]
        sel = np.nonzero(ld >= 0)[0]
        ldst = ld[sel]
        o = np.argsort(ldst, kind="stable")
        sel, ldst = sel[o], ldst[o]
        per_core_edges.append((sel, ldst))
        bstarts = np.searchsorted(ldst, np.arange(0, (nb1 + 1) * BLK, BLK))
        for b in range(nb1):
            eb = sel[bstarts[b]:bstarts[b + 1]]
            db = ldst[bstarts[b]:bstarts[b + 1]] - b * BLK
            hh = src[eb] >= half_nodes
            for h in (0, 1):
                m = hh == bool(h)
                streams[(core, b, h)] = (eb[m], db[m])
                lens.append(int(m.sum()))
    NI = max(64, -(-max(lens) // 64) * 64)  # idxs per (block, half), %64
    NIc = NI // 64
    cpb1 = 2 * NIc                          # transposed chunks per block
    n1_chunks = nb1 * cpb1

    # idxg: [cores, 128, nb1*NI/16] int16 -- half h on partitions h*64..+64,
    # replicated per 16-partition group, rank i at (partition i%16, col i//16)
    idxg = np.zeros((N_CORES, 128, nb1 * (NI // 16)), np.int16)
    dm1 = np.full((N_CORES, n1_chunks, CHUNK), -1, np.int64)
    wv1 = np.zeros((N_CORES, n1_chunks, CHUNK), np.float32)
    for core in range(N_CORES):
        for b in range(nb1):
            cols = slice(b * (NI // 16), (b + 1) * (NI // 16))
            for h in (0, 1):
                ee, dd = streams[(core, b, h)]
                r = src[ee] % half_nodes
                pairs, par = r // 2, r % 2
                arr = np.zeros(NI, np.int16)
                arr[:len(ee)] = pairs
                wrap = arr.reshape(NI // 16, 16).T      # [16, NI/16]
                for g in range(4):
                    idxg[core, h * 64 + g * 16:h * 64 + (g + 1) * 16, cols] = wrap
                ji = np.arange(len(ee))
                cs = b * cpb1 + h * NIc + ji // 64
                rows = (ji % 64) * 2 + par
                dm1[core, cs, rows] = dd
                wv1[core, cs, rows] = w_edge[ee]

    def windows(dm, wv, nb, cpb, n_chunks):
        """Uniform narrow-window widths/offsets; returns (W, woff, Mfull, Mnarrow)."""
        lo = np.full(n_chunks, 10 ** 9, np.int64)
        hi = np.full(n_chunks, -1, np.int64)
        for c in range(n_chunks):
            v = dm[:, c][dm[:, c] >= 0]
            if len(v):
                lo[c], hi[c] = v.min(), v.max()
        narrow = np.arange(n_chunks) % cpb != 0
        spans = (hi - lo + 1)[narrow & (hi >= 0)]
        mspan = int(spans.max()) if len(spans) else 1
        Ww = int(min(BLK, max(8, 1 << int(np.ceil(np.log2(max(1, mspan)))))))
        Mfull = np.zeros((N_CORES, nb, CHUNK, BLK), np.float32)
        Mnarrow = np.zeros((N_CORES, nb, max(0, cpb - 1), CHUNK, Ww), np.float32)
        woff = np.zeros(nb * max(0, cpb - 1), np.int64)
        for b in range(nb):
            for jb in range(cpb):
                c = b * cpb + jb
                if jb > 0:
                    off = 0 if hi[c] < 0 else min(int(lo[c]), BLK - Ww)
                    woff[b * (cpb - 1) + jb - 1] = off
                for core in range(N_CORES):
                    valid = dm[core, c] >= 0
                    if not valid.any():
                        continue
                    rows = np.nonzero(valid)[0]
                    cols = dm[core, c][rows]
                    if jb == 0:
                        Mfull[core, b, rows, cols] = wv[core, c][rows]
                    else:
                        rel = cols - woff[b * (cpb - 1) + jb - 1]
                        if (rel < 0).any() or (rel >= Ww).any():
                            raise ValueError("window overflow")
                        Mnarrow[core, b, jb - 1, rows, rel] = wv[core, c][rows]
        return Ww, woff, Mfull, Mnarrow

    W1w, woff1, M1f, M1n = windows(dm1, wv1, nb1, cpb1, n1_chunks)

    # ---- layer-2 chunks (srcs -> local h1 rows, dsts -> local pool slots) ----
    cnt2 = np.zeros((N_CORES, npb), np.int64)
    pc_e2 = []
    for core in range(N_CORES):
        ee = e2[e2core == core]
        g = dst[ee] // BLK
        lb = g // N_CORES
        lslot = lb * BLK + dst[ee] % BLK
        o = np.argsort(lslot, kind="stable")
        ee, lslot = ee[o], lslot[o]
        pc_e2.append((ee, lslot))
        cnt2[core] = np.bincount(lslot // BLK, minlength=npb)
    cpb2 = int(max(1, -(-cnt2.max() // CHUNK)))
    n2_chunks = npb * cpb2

    idx2 = np.zeros((N_CORES, n2_chunks, CHUNK), np.int32)
    dm2 = np.full((N_CORES, n2_chunks, CHUNK), -1, np.int64)
    wv2 = np.zeros((N_CORES, n2_chunks, CHUNK), np.float32)
    for core in range(N_CORES):
        ee, lslot = pc_e2[core]
        bstarts = np.searchsorted(lslot, np.arange(0, (npb + 1) * BLK, BLK))
        for b in range(npb):
            eb = ee[bstarts[b]:bstarts[b + 1]]
            db = lslot[bstarts[b]:bstarts[b + 1]] - b * BLK
            for jb in range(cpb2):
                c = b * cpb2 + jb
                seg = slice(jb * CHUNK, (jb + 1) * CHUNK)
                es, dd = eb[seg], db[seg]
                if len(es) == 0:
                    continue
                idx2[core, c, :len(es)] = lut_ids[core][src[es]]
                dm2[core, c, :len(es)] = dd
                wv2[core, c, :len(es)] = w_edge[es]

    W2w, woff2, M2f, M2n = windows(dm2, wv2, npb, cpb2, n2_chunks)

    wts = np.zeros((65, 3 * 64), np.float32)
    wts[0:64, 0:64] = W1
    wts[0:64, 64:128] = W2
    wts[0:64, 128:192] = Wl
    wts[64, 0:64] = b1
    wts[64, 64:128] = b2

    mskv = np.zeros((N_CORES, 128, npb), np.float32)
    for core in range(N_CORES):
        for j in range(npb):
            g = core + j * N_CORES
            base = g * BLK
            n_valid = min(BLK, max(0, pool_n - base))
            mskv[core, :n_valid, j] = 1.0 / pool_n

    meta = {
        "n_nodes": n_nodes, "nb1": nb1, "NI": NI, "num_elems": num_elems,
        "W1w": W1w, "woff1": tuple(int(x) for x in woff1),
        "npb": npb, "cpb2": cpb2,
        "W2w": W2w, "woff2": tuple(int(x) for x in woff2),
        "bf16": bf16,
    }
    if bf16:
        import ml_dtypes
        ddt = ml_dtypes.bfloat16
    else:
        ddt = np.float32
    featf = np.ascontiguousarray(feat, dtype=np.float32)
    fp = np.zeros((128, num_elems * 2), np.float32)
    fp[0:64, :half_nodes] = featf[:half_nodes].T
    fp[64:128, :n_nodes - half_nodes] = featf[half_nodes:].T
    featp_host = fp.astype(ddt)
    in_maps = []
    for core in range(N_CORES):
        im = {
            "featp": featp_host,
            "idxg": np.ascontiguousarray(idxg[core]),
            "m1f": np.ascontiguousarray(
                M1f[core].transpose(1, 0, 2).reshape(CHUNK, nb1 * BLK)).astype(ddt),
            "idx2": np.ascontiguousarray(idx2[core].T),
            "m2f": np.ascontiguousarray(
                M2f[core].transpose(1, 0, 2).reshape(CHUNK, npb * BLK)).astype(ddt),
            "wts": wts,
            "msk": mskv[core],
        }
        if cpb1 > 1:
            im["m1n"] = np.ascontiguousarray(
                M1n[core].transpose(2, 0, 1, 3).reshape(
                    CHUNK, nb1 * (cpb1 - 1) * W1w)).astype(ddt)
        if cpb2 > 1:
            im["m2n"] = np.ascontiguousarray(
                M2n[core].transpose(2, 0, 1, 3).reshape(
                    CHUNK, npb * (cpb2 - 1) * W2w)).astype(ddt)
        in_maps.append(im)
    return meta, in_maps


def kernel(src, dst, feat, W1, b1, W2, b2, Wl, bl, order):
    from concourse.bass_utils import run_bass_kernel_spmd

    meta, in_maps = _prep(src, dst, feat, W1, b1, W2, b2, Wl, bl, order)
    key = tuple(sorted((k, v) for k, v in meta.items() if k != "woff1")) + (
        meta["woff1"], meta["woff2"])
    nc = _cache.get(key)
    if nc is None:
        nc = _build(meta)
        _cache[key] = nc
    last_err = None
    for _ in range(3):
        try:
            res = run_bass_kernel_spmd(nc, in_maps, core_ids=list(range(N_CORES)))
            parts = [np.asarray(res.results[c]["out"], dtype=np.float64)
                     for c in range(N_CORES)]
            return (np.sum(parts, axis=0) + np.asarray(bl, np.float64)).astype(
                np.float32)
        except Exception as e:  # transient terminal/runtime failures
            last_err = e
    raise last_err



# revision 3
# speedup vs baseline: 2.1127x; 2.1127x over previous
"""2-layer GCN (GraphConv x2 + mean-pool + linear) on 8 TRN2 NeuronCores, v3.3.

Dead-code-eliminated 1D partition: only h2 rows 0..order matter; each core
owns one 128-row pool block, the ~1.6k unique layer-2 srcs form its compact
h1 slot space (nb1 128-slot blocks), and layer 1 aggregates only the ~20k
edges per core incident to those slots.

All gathers run through gpsimd.dma_gather (InstDMAGatherAnt: vectorized SWDGE
descriptor generation, ~1.7ns/idx measured) in <=1024-idx ops (HW descriptor
ring limit) spread across the 4 SWDGE queues (matching Tile's DMASW lane
rotation; single-queue runs hit ring backpressure ~2.3x slower). Rows are
gathered 128B-narrow (elem 64 bf16, 256B stride) from DRAM featb [100k, 128]
straight into SBUF X tiles in edge order - no transposes, no PSUM staging.

Layer-1 edges are bucketed by (block-pair group g, int16 sub-table q) and
chunked 128 rows per matmul; chunks may span the two blocks of their group
(per-block segment matmuls acc_b[64ch, W] += X_chunk^T @ M_seg; a block's
first segment uses the full 128-wide window so its start=True zeroes the
whole PSUM region). Group locality lets PE consume gathers while later
groups are still fetching. Block epilogue: aug(+ones) @ W1aug -> z[128,64]
(bias via ones row), Lrelu on ACT, DMA to DRAM h1d (256B rows). Layer-2
gathers h1d rows by compact slot (same machinery, 2 ops), acc2 -> W2aug ->
Lrelu with accum_out giving the pooled sum; Wl is pre-scaled by 1/pool_n;
per-core partial [64] outputs summed on host (+ bl).
"""

import numpy as np

N_NODES = 100_000
C = 64
N_CORES = 8
NEG_SLOPE = 0.01
BLK = 128
CHUNK = 128        # edges per matmul chunk
OPCH = 8           # chunks per dma_gather op (1024 idxs = HW ring limit)
QS = 32768         # rows per sub-table (int16 idx space)
G = 2              # dst blocks per gather group (PSUM bank budget)

_cache = {}


def _pow2ceil(x):
    return 1 << int(np.ceil(np.log2(max(1, x))))


def _dma_gather_narrow(gp, out_ap, in_ap, idxs_ap, num_idxs, elem_size,
                       elem_step, queue_num=0):
    """Non-transpose HBM dma_gather with elem_size_bytes % 256 != 0 (bass's
    %256 assert is a transpose-mode restriction; ucode+sim handle narrow
    elems with elem_step providing the 256B-aligned row stride)."""
    import concourse.mybir as mybir
    from concourse import ap_utils
    from concourse.bass import MemorySpace, exact_div

    assert in_ap.space == MemorySpace.DRAM
    assert idxs_ap.space == MemorySpace.SBUF
    assert out_ap.space == MemorySpace.SBUF
    assert idxs_ap.dtype == mybir.dt.int16
    assert in_ap.dtype == out_ap.dtype
    assert ap_utils.ap_is_contiguous(out_ap.ap[1:])
    assert ap_utils.ap_is_contiguous(idxs_ap.ap[1:])
    assert in_ap.ap[-1][1] == out_ap.ap[-1][1] == elem_size
    assert out_ap.ap[0][1] * out_ap.ap[1][1] == ((num_idxs + 127) // 128) * 128
    assert in_ap.ap[0][0] == elem_step
    stride_bytes = elem_step * mybir.dt.size(in_ap.dtype)
    return gp.add_instruction(
        mybir.InstDMAGatherAnt(
            name=gp.bass.get_next_instruction_name(),
            ins=[*gp.lower_ap_dma(in_ap, for_custom_bir_dma=True),
                 gp.lower_ap(idxs_ap),
                 gp.lower_val_access(gp.to_reg(num_idxs))],
            outs=[gp.lower_ap(out_ap)],
            transpose=False, num_idxs=num_idxs, elem_size=elem_size,
            stride_bytes_256=exact_div(stride_bytes, 256),
            gen_mode=0, single_packet=True, queue_num=queue_num,
            sbuf_tokens_per_rank=0, sbuf_free_dim_per_rank=0,
            sbuf_free_dim_pad_per_rank=0, sbuf_byte_offset=0))


def _build(meta):
    import concourse.bass as bass
    import concourse.bacc as bacc
    import concourse.mybir as mybir
    import concourse.tile as tile

    f32 = mybir.dt.float32
    i16 = mybir.dt.int16
    dt = mybir.dt.bfloat16

    nb1 = meta["nb1"]
    nch1 = meta["nch1"]
    segs1 = meta["segs1"]      # per chunk: tuple of (b, off, w, mcol, st, sp)
    ops1 = meta["ops1"]        # (q, chunk_start, nchunks) per gather op
    consume = meta["consume"]  # chunk consumption (matmul) order
    emit_order = meta["emit_order"]
    m1cols = meta["m1cols"]
    nch2 = meta["nch2"]
    woff2 = meta["woff2"]
    cw2 = meta["cw2"]
    cs2 = meta["cs2"]
    ops2 = meta["ops2"]        # (chunk_start, nchunks)
    m2cols = cs2[-1] + cw2[-1]
    reps = meta.get("reps", 1)
    actf = meta.get("act", "Lrelu")
    parts = meta.get("parts", "full")

    nc = bacc.Bacc(None, target_bir_lowering=False, num_swdge_queues=4)

    featb = nc.declare_dram_parameter("featb", [N_NODES, 128], dt,
                                      isOutput=False)
    idx1 = nc.declare_dram_parameter("idx1", [128, nch1 * 8], i16,
                                     isOutput=False)
    m1 = nc.declare_dram_parameter("m1", [128, m1cols], dt, isOutput=False)
    idx2 = nc.declare_dram_parameter("idx2", [128, nch2 * 8], i16,
                                     isOutput=False)
    m2 = nc.declare_dram_parameter("m2", [128, m2cols], dt, isOutput=False)
    wts = nc.declare_dram_parameter("wts", [65, 3 * 64], f32, isOutput=False)
    outp = nc.declare_dram_parameter("out", [64], f32, isOutput=True)

    chunk_op = np.zeros(nch1, np.int64)
    for opi, (q, cs, nch) in enumerate(ops1):
        chunk_op[cs:cs + nch] = opi

    with tile.TileContext(nc) as tc:
        with (
            tc.tile_pool(name="dram", bufs=1, space="DRAM") as dram,
            tc.tile_pool(name="res", bufs=1) as res,
            tc.tile_pool(name="xb", bufs=len(ops1) + 2) as xpool,
            tc.tile_pool(name="x2b", bufs=2) as x2pool,
            tc.tile_pool(name="ep", bufs=4) as ep,
            tc.tile_pool(name="psA", bufs=4, space="PSUM") as psA,
            tc.tile_pool(name="psB", bufs=2, space="PSUM") as psB,
            tc.tile_pool(name="psC", bufs=1, space="PSUM") as psC,
        ):
            h1d = dram.tile([nb1 * BLK, 128], dt)
            idx1_t = res.tile([128, nch1 * 8], i16)
            m1_t = res.tile([128, m1cols], dt)
            idx2_t = res.tile([128, nch2 * 8], i16)
            m2_t = res.tile([128, m2cols], dt)
            wts_t = res.tile([65, 3 * 64], f32)
            nc.sync.dma_start(out=idx1_t[:], in_=idx1[:, :])
            nc.sync.dma_start(out=m1_t[:], in_=m1[:, :])
            nc.sync.dma_start(out=idx2_t[:], in_=idx2[:, :])
            nc.sync.dma_start(out=m2_t[:], in_=m2[:, :])
            nc.sync.dma_start(out=wts_t[:], in_=wts[:, :])
            w1a = res.tile([65, 64], dt)
            w2a = res.tile([65, 64], dt)
            wl_t = res.tile([64, 64], f32)
            nc.vector.tensor_copy(out=w1a[:], in_=wts_t[:, 0:64])
            nc.vector.tensor_copy(out=w2a[:], in_=wts_t[:, 64:128])
            nc.vector.tensor_copy(out=wl_t[:], in_=wts_t[0:64, 128:192])

            swdge_ctr = [0]

            def body():
                def epilogue(b, acc, ei):
                    aug = ep.tile([65, BLK], dt, tag="aug")
                    if ei % 2:
                        nc.scalar.activation(
                            out=aug[0:64, :], in_=acc[:, :],
                            func=mybir.ActivationFunctionType.Copy)
                    else:
                        nc.vector.tensor_copy(out=aug[0:64, :], in_=acc[:, :])
                    nc.vector.memset(aug[64:65, :], 1.0)
                    z = psB.tile([BLK, 64], f32, tag="z")
                    nc.tensor.matmul(out=z[:, :], lhsT=aug[:], rhs=w1a[:],
                                     start=True, stop=True)
                    hs = ep.tile([BLK, 64], dt, tag="hs")
                    nc.scalar.activation(
                        out=hs[:, :], in_=z[:, :],
                        func=getattr(mybir.ActivationFunctionType, actf),
                        scale=1.0, alpha=NEG_SLOPE)
                    nc.sync.dma_start(out=h1d[b * BLK:(b + 1) * BLK, 0:64],
                                      in_=hs[:])

                xtiles = {}
                for opi in emit_order:
                    q, cs, nch = ops1[opi]
                    x = xpool.tile([128, nch, 64], dt, tag="x",
                                   name=f"x{opi}")
                    rows = min(QS, N_NODES - q * QS)
                    _dma_gather_narrow(
                        nc.gpsimd, x[:, :, :],
                        featb[q * QS:q * QS + rows, 0:64],
                        idx1_t[:, cs * 8:(cs + nch) * 8],
                        nch * 128, 64, 128,
                        queue_num=swdge_ctr[0] % 4)
                    swdge_ctr[0] += 1
                    xtiles[opi] = (x, cs)

                if parts == "gather":
                    ofin = ep.tile([64, 1], f32, tag="ofin")
                    nc.vector.memset(ofin[:], 0.0)
                    nc.sync.dma_start(out=outp[:, None], in_=ofin[:])
                    return

                accs = {}
                ei = 0
                for c in consume:
                    x, cs = xtiles[int(chunk_op[c])]
                    for (b, off, w, mcol, st, sp) in segs1[c]:
                        if st:
                            accs[b] = psA.tile([64, BLK], f32, tag="acc",
                                               name=f"acc_{b}")
                        nc.tensor.matmul(
                            out=accs[b][:, off:off + w],
                            lhsT=x[:, c - cs, :],
                            rhs=m1_t[:, mcol:mcol + w],
                            start=bool(st), stop=bool(sp))
                        if sp:
                            epilogue(b, accs.pop(b), ei)
                            ei += 1

                if parts == "l1":
                    ofin = ep.tile([64, 1], f32, tag="ofin")
                    nc.vector.memset(ofin[:], 0.0)
                    nc.sync.dma_start(out=outp[:, None], in_=ofin[:])
                    return

                # ---- layer 2 ----
                x2s = []
                for (opi2, (cs, nch)) in enumerate(ops2):
                    x2 = x2pool.tile([128, nch, 64], dt, tag="x2")
                    _dma_gather_narrow(
                        nc.gpsimd, x2[:, :, :], h1d[:, 0:64],
                        idx2_t[:, cs * 8:(cs + nch) * 8],
                        nch * 128, 64, 128,
                        queue_num=swdge_ctr[0] % 4)
                    swdge_ctr[0] += 1
                    x2s.append((cs, x2))
                acc2 = psC.tile([64, BLK], f32, tag="acc2")
                for (opi, (cs, x2)) in enumerate(x2s):
                    nch_op = ops2[opi][1]
                    for k in range(nch_op):
                        c = cs + k
                        nc.tensor.matmul(
                            out=acc2[:, woff2[c]:woff2[c] + cw2[c]],
                            lhsT=x2[:, k, :],
                            rhs=m2_t[:, cs2[c]:cs2[c] + cw2[c]],
                            start=(c == 0), stop=(c == nch2 - 1))
                aug2 = ep.tile([65, BLK], dt, tag="aug2")
                nc.vector.tensor_copy(out=aug2[0:64, :], in_=acc2[:, :])
                nc.vector.memset(aug2[64:65, :], 1.0)
                z2 = psB.tile([64, BLK], f32, tag="z")
                nc.tensor.matmul(out=z2[:, :], lhsT=w2a[:], rhs=aug2[:],
                                 start=True, stop=True)
                h2s = ep.tile([64, BLK], dt, tag="h2s")
                pooled = ep.tile([64, 1], f32, tag="pooled")
                nc.scalar.activation(
                    out=h2s[:, :], in_=z2[:, :],
                    func=getattr(mybir.ActivationFunctionType, actf),
                    scale=1.0, alpha=NEG_SLOPE, accum_out=pooled[:, 0:1])
                zf = psC.tile([64, 1], f32, tag="zf")
                nc.tensor.matmul(out=zf[:, :], lhsT=wl_t[:], rhs=pooled[:],
                                 start=True, stop=True)
                ofin = ep.tile([64, 1], f32, tag="ofin")
                nc.vector.tensor_copy(out=ofin[:], in_=zf[:, :])
                nc.sync.dma_start(out=outp[:, None], in_=ofin[:])

            for _ in range(reps):
                body()

    # Align each gather's SWDGE queue with the DMASW sem lane Tile assigned
    # (lane L is serviced by queue L % 4); Tile assigns lanes in scheduled
    # order, which need not match emission order.
    from concourse.tile_sem_assignment import PROC_NAME_TO_IDX
    lane_of = {v: int(k[5:]) for k, v in PROC_NAME_TO_IDX.items()
               if k.startswith("DMASW")}
    for blk in nc.main_func.blocks:
        for inst in blk.instructions:
            if isinstance(inst, mybir.InstDMAGatherAnt):
                proc = getattr(inst, "bass_scheduled_proc", None)
                if proc in lane_of:
                    inst.queue_num = lane_of[proc] % 4

    nc.compile()
    return nc


def _wrap_idxs(vals):
    """[n*128] int16 -> [128, n*8] wrapped (pos j -> partition j%16, col j//16,
    replicated across the 8 16-partition groups)."""
    n = len(vals) // 128
    cols = vals.reshape(n * 8, 16).T        # [16, n*8]
    return np.tile(cols, (8, 1)).astype(np.int16)


def _even_chunks(n, nchunks):
    """Spread n sorted items over nchunks chunks evenly; returns (chunk_id,
    row_within_chunk) per item."""
    if n == 0:
        return np.zeros(0, np.int64), np.zeros(0, np.int64)
    pos = np.arange(n)
    ck = (pos * nchunks) // n
    counts = np.bincount(ck, minlength=nchunks)
    starts = np.concatenate([[0], np.cumsum(counts)[:-1]])
    row = pos - starts[ck]
    assert row.max() < CHUNK
    return ck, row


def _prep(src, dst, feat, W1, b1, W2, b2, Wl, bl, order):
    src = np.asarray(src).astype(np.int64)
    dst = np.asarray(dst).astype(np.int64)
    n_nodes = feat.shape[0]
    pool_n = int(order) + 1
    assert pool_n == BLK * N_CORES, pool_n

    out_deg = np.maximum(np.bincount(src, minlength=n_nodes), 1)
    in_deg = np.maximum(np.bincount(dst, minlength=n_nodes), 1)
    o_is = (out_deg.astype(np.float64) ** -0.5).astype(np.float32)
    i_is = (in_deg.astype(np.float64) ** -0.5).astype(np.float32)
    w_edge = (o_is[src] * i_is[dst]).astype(np.float32)

    e2 = np.nonzero(dst < pool_n)[0]
    e2core = dst[e2] // BLK                  # block j -> core j (npb == 1)

    luts, kcs = [], []
    for core in range(N_CORES):
        uniq = np.unique(src[e2[e2core == core]])
        lut = np.full(n_nodes, -1, np.int64)
        lut[uniq] = np.arange(len(uniq))
        luts.append(lut)
        kcs.append(len(uniq))
    nb1 = -(-max(kcs) // BLK)
    NQ1 = -(-n_nodes // QS)
    NG = -(-nb1 // G)

    # ---- L1: per-core edges sorted by (g, q, b, dloc); shared chunks ----
    percore = []
    cnt = np.zeros((N_CORES, NG, NQ1), np.int64)
    for core in range(N_CORES):
        ld = luts[core][dst]
        sel = np.nonzero(ld >= 0)[0]
        l = ld[sel]
        b = l // BLK
        g = b // G
        dloc = l % BLK
        q = src[sel] // QS
        o = np.lexsort((dloc, b, q, g))
        sel, b, dloc, q, g = sel[o], b[o], dloc[o], q[o], g[o]
        percore.append((sel, b, dloc, q, g))
        np.add.at(cnt[core], (g, q), 1)
    cgq = -(-cnt.max(axis=0) // CHUNK)       # [NG, NQ1]

    chq, chg = [], []
    cbase = np.full((NG, NQ1), -1, np.int64)
    for q in range(NQ1):
        for g in range(NG):
            if cgq[g][q] > 0:
                cbase[g][q] = len(chq)
                chq += [q] * cgq[g][q]
                chg += [g] * cgq[g][q]
    nch1 = len(chq)
    chq = np.array(chq)
    chg = np.array(chg)
    consume = [cbase[g][q] + k for g in range(NG) for q in range(NQ1)
               if cgq[g][q] > 0 for k in range(cgq[g][q])]

    bmat = np.full((N_CORES, nch1, CHUNK), -1, np.int64)
    dmat = np.full((N_CORES, nch1, CHUNK), -1, np.int64)
    wmat = np.zeros((N_CORES, nch1, CHUNK), np.float32)
    imat = np.zeros((N_CORES, nch1, CHUNK), np.int64)
    for core in range(N_CORES):
        sel, b, dloc, q, g = percore[core]
        key = g * NQ1 + q
        bnd = np.searchsorted(key, np.arange(NG * NQ1 + 1))
        for kk in range(NG * NQ1):
            s, e = bnd[kk], bnd[kk + 1]
            if s == e:
                continue
            gg, qq = kk // NQ1, kk % NQ1
            ck, row = _even_chunks(e - s, cgq[gg][qq])
            gch = cbase[gg][qq] + ck
            imat[core, gch, row] = src[sel[s:e]] - qq * QS
            bmat[core, gch, row] = b[s:e]
            dmat[core, gch, row] = dloc[s:e]
            wmat[core, gch, row] = w_edge[sel[s:e]]

    # ---- per-chunk per-block segments (flags over the consume order) ----
    seen_first = set()
    seg_raw = {}          # chunk -> list of [b, off, w]
    for c in consume:
        blocks = np.unique(bmat[:, c][bmat[:, c] >= 0])
        segs = []
        for b in sorted(int(x) for x in blocks):
            rows_mask = bmat[:, c] == b
            d = dmat[:, c][rows_mask]
            lo, hi = int(d.min()), int(d.max())
            if b not in seen_first:
                seen_first.add(b)
                w, off = BLK, 0
            else:
                w = min(BLK, max(16, _pow2ceil(hi - lo + 1)))
                off = min(lo, BLK - w)
            segs.append([b, off, w])
        seg_raw[c] = segs
    last_seg = {}
    for c in consume:
        for si, (b, off, w) in enumerate(seg_raw[c]):
            last_seg[b] = (c, si)
    first_emitted = set()
    segs1 = [None] * nch1
    mcol = 0
    M1list = []
    for c in consume:
        out = []
        for si, (b, off, w) in enumerate(seg_raw[c]):
            st = b not in first_emitted
            if st:
                first_emitted.add(b)
            sp = last_seg[b] == (c, si)
            out.append((int(b), int(off), int(w), int(mcol), int(st), int(sp)))
            Mseg = np.zeros((N_CORES, CHUNK, w), np.float32)
            rows_mask = bmat[:, c] == b
            for core in range(N_CORES):
                valid = np.nonzero(rows_mask[core])[0]
                if len(valid):
                    rel = dmat[core, c][valid] - off
                    assert (rel >= 0).all() and (rel < w).all()
                    np.add.at(Mseg[core], (valid, rel), wmat[core, c][valid])
            M1list.append(Mseg)
            mcol += w
        segs1[c] = tuple(out)
    m1cols = mcol
    M1 = np.concatenate(M1list, axis=2) if M1list else \
        np.zeros((N_CORES, CHUNK, 0), np.float32)

    ops1 = []
    c = 0
    while c < nch1:
        q = chq[c]
        e = c
        while e < nch1 and chq[e] == q and e - c < OPCH:
            e += 1
        ops1.append((int(q), int(c), int(e - c)))
        c = e
    emit_order = sorted(range(len(ops1)),
                        key=lambda i: (int(chg[ops1[i][1]]), ops1[i][0]))

    # ---- L2 ----
    cnt2 = np.zeros(N_CORES, np.int64)
    pc2 = []
    for core in range(N_CORES):
        ee = e2[e2core == core]
        slot = luts[core][src[ee]]
        dloc = dst[ee] % BLK
        o = np.argsort(dloc, kind="stable")
        ee, slot, dloc = ee[o], slot[o], dloc[o]
        pc2.append((ee, slot, dloc))
        cnt2[core] = len(ee)
    nch2 = int(-(-cnt2.max() // CHUNK))

    dmat2 = np.full((N_CORES, nch2, CHUNK), -1, np.int64)
    wmat2 = np.zeros((N_CORES, nch2, CHUNK), np.float32)
    imat2 = np.zeros((N_CORES, nch2, CHUNK), np.int64)
    for core in range(N_CORES):
        ee, slot, dloc = pc2[core]
        ck, row = _even_chunks(len(ee), nch2)
        imat2[core, ck, row] = slot
        dmat2[core, ck, row] = dloc
        wmat2[core, ck, row] = w_edge[ee]

    lo2 = np.full(nch2, 10 ** 9, np.int64)
    hi2 = np.full(nch2, -1, np.int64)
    for c in range(nch2):
        v = dmat2[:, c][dmat2[:, c] >= 0]
        if len(v):
            lo2[c], hi2[c] = v.min(), v.max()
    cw2 = np.array(
        [BLK if (c == 0 or hi2[c] < 0)
         else min(BLK, max(16, _pow2ceil(int(hi2[c] - lo2[c] + 1))))
         for c in range(nch2)], np.int64)
    cs2 = np.concatenate([[0], np.cumsum(cw2)[:-1]])
    woff2 = np.zeros(nch2, np.int64)
    M2 = np.zeros((N_CORES, CHUNK, int(cw2.sum())), np.float32)
    for c in range(nch2):
        if c > 0:
            woff2[c] = 0 if hi2[c] < 0 else min(int(lo2[c]), BLK - cw2[c])
        for core in range(N_CORES):
            valid = np.nonzero(dmat2[core, c] >= 0)[0]
            if len(valid) == 0:
                continue
            rel = dmat2[core, c][valid] - woff2[c]
            assert (rel >= 0).all() and (rel < cw2[c]).all()
            np.add.at(M2[core], (valid, cs2[c] + rel), wmat2[core, c][valid])

    ops2 = []
    c = 0
    while c < nch2:
        n = min(OPCH, nch2 - c)
        ops2.append((int(c), int(n)))
        c += n

    wts = np.zeros((65, 3 * 64), np.float32)
    wts[0:64, 0:64] = W1
    wts[64, 0:64] = b1
    wts[0:64, 64:128] = W2
    wts[64, 64:128] = b2
    wts[0:64, 128:192] = np.asarray(Wl, np.float32) / pool_n

    import ml_dtypes
    ddt = ml_dtypes.bfloat16
    featb = np.zeros((n_nodes, 128), np.float32)
    featb[:, 0:64] = np.asarray(feat, np.float32)
    featb = featb.astype(ddt)

    meta = {
        "nb1": nb1, "nch1": nch1,
        "segs1": tuple(segs1),
        "m1cols": int(m1cols),
        "ops1": tuple(ops1),
        "consume": tuple(int(x) for x in consume),
        "emit_order": tuple(int(x) for x in emit_order),
        "nch2": nch2, "woff2": tuple(int(x) for x in woff2),
        "cw2": tuple(int(x) for x in cw2),
        "cs2": tuple(int(x) for x in cs2),
        "ops2": tuple(ops2),
    }
    in_maps = []
    for core in range(N_CORES):
        im = {
            "featb": featb,
            "idx1": _wrap_idxs(imat[core].reshape(-1)),
            "m1": np.ascontiguousarray(M1[core]).astype(ddt),
            "idx2": _wrap_idxs(imat2[core].reshape(-1)),
            "m2": np.ascontiguousarray(M2[core]).astype(ddt),
            "wts": wts,
        }
        in_maps.append(im)
    return meta, in_maps


def kernel(src, dst, feat, W1, b1, W2, b2, Wl, bl, order):
    from concourse.bass_utils import run_bass_kernel_spmd

    meta, in_maps = _prep(src, dst, feat, W1, b1, W2, b2, Wl, bl, order)
    key = tuple(sorted((k, v) for k, v in meta.items()))
    nc = _cache.get(key)
    if nc is None:
        nc = _build(meta)
        _cache[key] = nc
    last_err = None
    for _ in range(3):
        try:
            res = run_bass_kernel_spmd(nc, in_maps, core_ids=list(range(N_CORES)))
            parts = [np.asarray(res.results[c]["out"], dtype=np.float64)
                     for c in range(N_CORES)]
            return (np.sum(parts, axis=0) + np.asarray(bl, np.float64)).astype(
                np.float32)
        except Exception as e:
            last_err = e
    raise last_err
